# revision 1
# baseline (speedup 1.0000x reference)
"""Trainium2 Bass kernel for nn_CaptioningRNN (attention LSTM over T=64 steps).

Strategy (8-way tensor-parallel over H / gate dim):
 - Core k owns h-slice hk = [128k, 128k+128) and gate columns
   cols_k = {1024*g + 128k .. +128 : g in i,f,o,g}.
 - A is sharded by h for scores (A1: [h_loc, n, l]) and by l for the
   attention readout (A2: [l, n, h_loc]); both bf16, resident in SBUF.
 - Per-sample attention einsums run on the PE as block-diagonal matmuls
   (zero-padded 32-col lhsT tiles, 4x concurrent via tile_position);
   diagonals extracted with mask-multiply + strided reduce on DVE.
 - Per step TWO collectives: AllGather(h^T chunk ++ scores partial) and
   AllGather(attn^T chunk); the scores AllReduce is folded into the
   first AG (partials summed on DVE after the gather).
 - x@Wx + b precomputed on host, streamed per step.
Numerics: bf16 matmul operands, fp32 PSUM/state/softmax.
"""

import os
from contextlib import ExitStack

import numpy as np
import ml_dtypes

import concourse.bass as bass
import concourse.tile as tile
from concourse import bacc, mybir
from concourse.bass_utils import run_bass_kernel_spmd
from concourse.masks import make_identity

F32 = mybir.dt.float32
BF16 = mybir.dt.bfloat16
AF = mybir.ActivationFunctionType
OP = mybir.AluOpType

N, T, D, H = 128, 64, 512, 1024
L = 196
NCORES = 8
HS = H // NCORES          # 128
CS = 4 * H // NCORES      # 512
SCALE = 1.0 / float(np.sqrt(H))
G = 4                     # readout block-diag sample group size
NG = N // G               # 32 readout groups
GS = 2                    # scores block-diag sample group size
NGS = N // GS             # 64 scores groups
CAT = 128 + L             # combined AG payload cols (h^T ++ scores partial)

TSTEPS = int(os.environ.get("KERNEL_TSTEPS", T))
PACK = bool(int(os.environ.get("KERNEL_PACK", "1")))
NOCOLL = bool(int(os.environ.get("KERNEL_NOCOLL", "0")))
REPEAT = int(os.environ.get("KERNEL_REPEAT", "1"))
DMAQ_SPREAD = False
RG = [list(range(NCORES))]


def _ap(t, dims, offset=0):
    a = t[:]
    return bass.AP(a.tensor, a.offset + offset, [a.ap[0]] + dims)


def build_nc(tsteps):
    nc = bacc.Bacc("TRN2", target_bir_lowering=False, debug=False,
                   num_devices=NCORES)
    d_a1 = nc.dram_tensor("a1", (128, N, L), BF16, kind="ExternalInput").ap()
    d_a2 = nc.dram_tensor("a2", (2, 128, N, HS), BF16, kind="ExternalInput").ap()
    d_w = nc.dram_tensor("wslab", (128, 16, CS), BF16, kind="ExternalInput").ap()
    d_xwb = nc.dram_tensor("xwb", (tsteps, N, CS), F32, kind="ExternalInput").ap()
    d_sm = nc.dram_tensor("smask", (128, GS * L), F32, kind="ExternalInput").ap()
    d_rm = nc.dram_tensor("rmask", (128, G * HS), F32, kind="ExternalInput").ap()
    d_out = nc.dram_tensor("hout", (tsteps, N, HS), BF16, kind="ExternalOutput").ap()

    with tile.TileContext(nc) as tc:
        with ExitStack() as ctx:
            _build(ctx, tc, tsteps, d_a1, d_a2, d_w, d_xwb, d_sm, d_rm, d_out)
    nc.compile()
    return nc


def _build(ctx, tc, tsteps, d_a1, d_a2, d_w, d_xwb, d_sm, d_rm, d_out):
    nc = tc.nc
    pp = ctx.enter_context(tc.tile_pool(name="persist", bufs=1))
    sb = ctx.enter_context(tc.tile_pool(name="work", bufs=3))
    ps_s = ctx.enter_context(tc.tile_pool(name="ps_s", bufs=1, space="PSUM"))
    ps_r = ctx.enter_context(tc.tile_pool(name="ps_r", bufs=1, space="PSUM"))
    ps_g = ctx.enter_context(tc.tile_pool(name="ps_g", bufs=2, space="PSUM"))
    ps_t = ctx.enter_context(tc.tile_pool(name="ps_t", bufs=1, space="PSUM"))
    dr = ctx.enter_context(tc.tile_pool(name="bounce", bufs=4, space="DRAM"))

    # ---- persistent tiles
    t_a1 = pp.tile([128, N * L], BF16)           # [h_loc | n, l]
    t_a2 = pp.tile([128, 2 * N * HS], BF16)      # [l_loc | lc, n, h_loc]
    t_w = pp.tile([128, 16 * CS], BF16)          # [hrow | chunk, col]
    t_sm = pp.tile([128, GS * L], F32)
    t_rm = pp.tile([128, G * HS], F32)
    # block-diag lhsT tiles: 32-col per group, few live cols each
    t_hbd = pp.tile([128, NGS * 32], BF16)
    t_wbd0 = pp.tile([128, NG * 32], BF16)
    t_wbd1 = pp.tile([128, NG * 32], BF16)
    t_hg = pp.tile([128, NCORES * 128], BF16)    # gathered h^T chunks
    t_sg = pp.tile([128, NCORES * L], BF16)      # gathered scores partials
    t_aTg = pp.tile([128, NCORES * 128], BF16)   # gathered attn^T
    t_c = pp.tile([128, HS], F32)
    t_wbf = pp.tile([128, 256], BF16)            # exp weights (196 used)
    t_id = pp.tile([128, 128], BF16)
    t_idf = pp.tile([128, 128], F32)

    # ---- loads
    nc.sync.dma_start(t_a1[:], d_a1.rearrange("p n l -> p (n l)"))
    nc.sync.dma_start(_ap(t_a2, [[N * HS, 2], [HS, N], [1, HS]]),
                      d_a2.rearrange("c p n h -> p c n h"))
    nc.sync.dma_start(t_w[:], d_w.rearrange("p c x -> p (c x)"))
    nc.sync.dma_start(t_sm[:], d_sm)
    nc.sync.dma_start(t_rm[:], d_rm)
    make_identity(nc, t_id[:])
    make_identity(nc, t_idf[:])
    nc.vector.memset(t_hbd[:], 0.0)
    nc.vector.memset(t_wbf[:], 0.0)
    nc.vector.memset(t_wbd0[:], 0.0)
    nc.vector.memset(t_wbd1[:], 0.0)

    # block-diag fills. Sample s = 32a + G*b + j (block a, group-in-block b,
    # lane j) lands at dst col q*32 + (s - 32a) with q the global group id.
    def fill_bd_g(dst_tile, src_ap128, g, rows=128):
        npb = 32 // g                      # groups per 32-sample block
        src = bass.AP(src_ap128.tensor, src_ap128.offset,
                      [[src_ap128.ap[0][0], rows], [32, 4], [g, npb], [1, g]])
        d = _ap(dst_tile, [[npb * 32, 4], [32 + g, npb], [1, g]])
        dst = bass.AP(d.tensor, d.offset, [[d.ap[0][0], rows]] + d.ap[1:])
        nc.vector.tensor_copy(dst, src)

    def fill_bd(dst_tile, src_ap128, rows=128):      # readout (G=4)
        fill_bd_g(dst_tile, src_ap128, G, rows)

    def fill_bd_s(dst_tile, src_ap128, rows=128):    # scores (GS=2)
        fill_bd_g(dst_tile, src_ap128, GS, rows)

    # ---- scores partial: 64 MMs of 392 cols (4x col-packed)
    def scores_block(s_ps):
        for jc in range(4):
            tp = (0, 32 * jc) if PACK else None
            orng = slice(32 * jc, 32 * jc + 32) if PACK else slice(0, 128)
            for qq in range(16):
                q = 16 * jc + qq
                base = q * GS * L
                lhs = t_hbd[:, q * 32:(q + 1) * 32]
                nc.tensor.matmul(s_ps[orng, 0:GS * L], lhs,
                                 t_a1[:, base:base + GS * L],
                                 start=(qq == 0), stop=(qq == 15),
                                 tile_position=tp)

    def extract_scores(s_ps):
        stmp = sb.tile([128, GS * L], F32)
        nc.vector.tensor_tensor(stmp[:], s_ps[:, 0:GS * L], t_sm[:],
                                op=OP.mult)
        sc = sb.tile([128, L], BF16, tag="scb")
        with nc.allow_low_precision(reason="2-way partial-score sum; "
                                    "summed again in f32 after the gather"):
            nc.vector.tensor_reduce(sc[:], _ap(stmp, [[1, L], [L, GS]]),
                                    axis=mybir.AxisListType.X, op=OP.add)
        return sc

    def ag_to(src_tile, dst_tile, cols, tr=False, tag=""):
        """AllGather a (128, cols) bf16 tile into dst[128, 8*cols].
        tr=True: transpose each gathered (128, cols) chunk during the
        unload DMA (XBAR), so dst chunk c = src_c^T."""
        bi = dr.tile([128, cols], BF16, tag=f"bi{cols}{tag}")
        nc.sync.dma_start(bi[:], src_tile)
        if NOCOLL:
            src = bi
        else:
            bo = dr.tile([NCORES * 128, cols], BF16, tag=f"bo{cols}{tag}")
            nc.gpsimd.collective_compute("AllGather", OP.bypass, ins=[bi.opt()],
                                         outs=[bo.opt()], replica_groups=RG)
            src = bo
        if tr and not NOCOLL:
            # [8*128, 128]^T = [128, 8*128] == per-chunk transposes, fused
            nc.scalar.dma_start_transpose(dst_tile[:], src[:])
        elif tr:
            for c in range(NCORES):
                nc.scalar.dma_start_transpose(
                    dst_tile[:, c * 128:(c + 1) * 128], src[:])
        elif NOCOLL:
            for c in range(NCORES):
                nc.gpsimd.dma_start(dst_tile[:, c * cols:(c + 1) * cols], bi[:])
        else:
            nc.sync.dma_start(_ap(dst_tile, [[cols, NCORES], [1, cols]]),
                              src[:].rearrange("(c p) x -> p c x", c=NCORES))

    # ---- init: h0 = mean_l A1
    h0raw = sb.tile([128, N], F32)
    nc.vector.tensor_reduce(h0raw[:], _ap(t_a1, [[L, N], [1, L]]),
                            axis=mybir.AxisListType.X, op=OP.add)
    h0T = sb.tile([128, N], BF16)
    nc.scalar.activation(h0T[:], h0raw[:], AF.Copy, scale=1.0 / L)
    fill_bd_s(t_hbd, h0T[:])
    tp0 = ps_t.tile([128, 128], F32, tag="tp0f")
    nc.tensor.transpose(tp0[:], h0raw[:], t_idf[:])
    nc.scalar.activation(t_c[:], tp0[:], AF.Copy, scale=1.0 / L)
    h0nb = sb.tile([128, 128], BF16)
    nc.scalar.activation(h0nb[:], tp0[:], AF.Copy, scale=1.0 / L)
    ag_to(h0nb[:], t_hg, 128, tr=True, tag="h")
    s_ps0 = ps_s.tile([128, 512], F32)
    scores_block(s_ps0)
    sc0 = extract_scores(s_ps0)
    ag_to(sc0[:], t_sg, L)

    for rep in range(REPEAT):
     for t in range(tsteps):
        # ===== gates h-part first: PE busy while DVE/ACT run the softmax
        g_ps = ps_g.tile([128, CS], F32)
        for c in range(NCORES):
            nc.tensor.matmul(g_ps[:], t_hg[:, c * 128:(c + 1) * 128],
                             t_w[:, c * CS:(c + 1) * CS],
                             start=(c == 0), stop=False)
        # ===== sum gathered scores partials, softmax (unnormalized: the
        # 1/sum(exp) factor is applied after the readout reduce instead,
        # keeping the reciprocal off the critical path)
        ssum = sb.tile([128, L], F32)
        nc.vector.tensor_reduce(ssum[:], _ap(t_sg, [[1, L], [L, NCORES]]),
                                axis=mybir.AxisListType.X, op=OP.add)
        esum = sb.tile([128, 1], F32)
        nc.scalar.activation(t_wbf[:, 0:L], ssum[:], AF.Exp, scale=SCALE,
                             accum_out=esum[:])
        rec = sb.tile([128, 1], F32)
        nc.vector.reciprocal(rec[:], esum[:])
        # ===== w^T via XBAR-transposing DMA -> block-diag lhsT
        for lc, (wbd, cols) in enumerate(((t_wbd0, 128), (t_wbd1, 68))):
            wT = sb.tile([128, 128], BF16, tag="wT")
            nc.sync.dma_start_transpose(
                wT[:], t_wbf[:, lc * 128:(lc + 1) * 128])
            fill_bd(wbd, wT[:], rows=cols)
        # ===== readout: attn (128n, 128h_loc), 64 MMs (4x packed)
        r_ps = ps_r.tile([128, G * HS], F32)
        for jc in range(4):
            tp = (0, 32 * jc) if PACK else None
            orng = slice(32 * jc, 32 * jc + 32) if PACK else slice(0, 128)
            for qq in range(8):
                q = 8 * jc + qq
                for lc, wbd in enumerate((t_wbd0, t_wbd1)):
                    base = lc * N * HS + q * G * HS
                    nc.tensor.matmul(r_ps[orng, :], wbd[:, q * 32:(q + 1) * 32],
                                     t_a2[:, base:base + G * HS],
                                     start=(qq == 0 and lc == 0),
                                     stop=(qq == 7 and lc == 1),
                                     tile_position=tp)
        rtmp = sb.tile([128, G * HS], F32)
        nc.vector.tensor_tensor(rtmp[:], r_ps[:], t_rm[:], op=OP.mult)
        attnf = sb.tile([128, HS], F32)
        nc.vector.tensor_reduce(attnf[:], _ap(rtmp, [[1, HS], [HS, G]]),
                                axis=mybir.AxisListType.X, op=OP.add)
        attnb = sb.tile([128, HS], BF16)
        nc.vector.tensor_scalar_mul(attnb[:], attnf[:], rec[:])
        # ===== AG2: attn (n, h_loc); chunks transposed during unload
        ag_to(attnb[:], t_aTg, 128, tr=True, tag="a")
        # ===== gates attn-part
        for c in range(NCORES):
            nc.tensor.matmul(g_ps[:], t_aTg[:, c * 128:(c + 1) * 128],
                             t_w[:, (8 + c) * CS:(9 + c) * CS],
                             start=False, stop=(c == NCORES - 1))
        xw = sb.tile([128, CS], F32)
        nc.scalar.dma_start(xw[:], d_xwb[t])
        asb = sb.tile([128, CS], F32)
        nc.vector.tensor_add(asb[:], g_ps[:], xw[:])
        # ===== pointwise LSTM
        sig = sb.tile([128, 384], F32)
        nc.scalar.activation(sig[:], asb[:, 0:384], AF.Sigmoid)
        tg = sb.tile([128, 128], F32)
        nc.scalar.activation(tg[:], asb[:, 384:512], AF.Tanh)
        c1 = sb.tile([128, 128], F32)
        nc.vector.tensor_mul(c1[:], sig[:, 128:256], t_c[:])
        c2 = sb.tile([128, 128], F32)
        nc.vector.tensor_mul(c2[:], sig[:, 0:128], tg[:])
        nc.vector.tensor_add(t_c[:], c1[:], c2[:])
        tch = sb.tile([128, 128], F32)
        nc.scalar.activation(tch[:], t_c[:], AF.Tanh)
        hf = sb.tile([128, 128], BF16)
        nc.vector.tensor_mul(hf[:], sig[:, 256:384], tch[:])
        nc.scalar.dma_start(d_out[t], hf[:])
        # ===== next-step h AG (launched ASAP) + scores partial + its AG
        if t < tsteps - 1 or rep < REPEAT - 1:
            ag_to(hf[:], t_hg, 128, tr=True, tag="h")
            hT = sb.tile([128, 128], BF16, tag="hT")
            nc.sync.dma_start_transpose(hT[:], hf[:])
            fill_bd_s(t_hbd, hT[:])
            s_ps = ps_s.tile([128, 512], F32)
            scores_block(s_ps)
            scb = extract_scores(s_ps)
            ag_to(scb[:], t_sg, L)


# ---------------------------------------------------------------------------
# host side
# ---------------------------------------------------------------------------
_NC_CACHE = {}


def _get_nc(tsteps):
    key = (tsteps, PACK)
    if key not in _NC_CACHE:
        _NC_CACHE[key] = build_nc(tsteps)
    return _NC_CACHE[key]


def _bf(x):
    return x.astype(ml_dtypes.bfloat16)


def prepare_inputs(x, A, Wx, Wh, Wattn, b, tsteps):
    Af = A.reshape(N, H, L).astype(np.float32)
    xwb = (_bf(x.reshape(N * T, D)).astype(np.float32)
           @ _bf(Wx).astype(np.float32)).reshape(N, T, 4 * H) + b[None, None, :]

    smask = np.zeros((128, GS * L), np.float32)
    for n in range(128):
        smask[n, (n % GS) * L:(n % GS) * L + L] = 1.0
    rmask = np.zeros((128, G * HS), np.float32)
    for n in range(128):
        rmask[n, (n % G) * HS:(n % G + 1) * HS] = 1.0

    in_maps = []
    for k in range(NCORES):
        hk = slice(128 * k, 128 * (k + 1))
        cols = np.concatenate([np.arange(1024 * g + 128 * k,
                                         1024 * g + 128 * (k + 1))
                               for g in range(4)])
        a1 = _bf(Af[:, hk, :].transpose(1, 0, 2))
        a2t = Af[:, hk, :].transpose(2, 0, 1)
        a2 = np.zeros((2, 128, N, HS), ml_dtypes.bfloat16)
        a2[0] = _bf(a2t[0:128])
        a2[1, 0:68] = _bf(a2t[128:196])
        wsl = np.empty((128, 16, CS), ml_dtypes.bfloat16)
        for c in range(8):
            wsl[:, c, :] = _bf(Wh[128 * c:128 * (c + 1)][:, cols])
            wsl[:, 8 + c, :] = _bf(Wattn[128 * c:128 * (c + 1)][:, cols])
        in_maps.append({
            "a1": np.ascontiguousarray(a1),
            "a2": a2,
            "wslab": wsl,
            "xwb": np.ascontiguousarray(
                xwb[:, :tsteps, cols].transpose(1, 0, 2)).astype(np.float32),
            "smask": smask,
            "rmask": rmask,
        })
    return in_maps


def kernel(x, A, Wx, Wh, Wattn, b, _tsteps=None):
    tsteps = _tsteps or TSTEPS
    x = np.asarray(x, np.float32)
    A = np.asarray(A, np.float32)
    nc = _get_nc(tsteps)
    in_maps = prepare_inputs(x, A, np.asarray(Wx, np.float32),
                             np.asarray(Wh, np.float32),
                             np.asarray(Wattn, np.float32),
                             np.asarray(b, np.float32), tsteps)
    res = run_bass_kernel_spmd(nc, in_maps, core_ids=list(range(NCORES)))
    out = np.empty((N, tsteps, H), np.float32)
    for k in range(NCORES):
        out[:, :, 128 * k:128 * (k + 1)] = \
            res.results[k]["hout"].astype(np.float32).transpose(1, 0, 2)
    if tsteps == T:
        return out
    full = np.zeros((N, T, H), np.float32)
    full[:, :tsteps] = out
    return full

